# revision 1
# baseline (speedup 1.0000x reference)
"""Trainium2 Bass kernel for the CSA (channel-spatial attention) module.

Reference computation (per batch b):
    q = Wq @ x[b]            # [64, N]
    k = Wk @ x[b]            # [64, N]
    E[n, m] = sum_c q[c, n] * k[c, m]          # [N, N]
    A = softmax(E, axis=m)
    v = Wv @ x_h[b]          # [128, N]
    out[c, n] = sum_m v[c, m] * A[n, m]
    result = gamma * out + x_h[b]

Sharding: 8 cores = 4 batches x 2 query-halves. Each core holds full K/V for
its batch and a 2048-wide query chunk (flash-style: the [N, N] attention
matrix is never materialized in HBM).

Key transformations vs the naive mapping:
- Wk is folded into the query projection on the host:
  E^T[m, n] = sum_c' xb[c', m] * qk[c', n]  with  qk = (Wk^T Wq) @ x_chunk,
  so K needs no on-chip projection and the energy matmul consumes DMA'd
  x directly as its stationary operand.
- Energy is computed transposed, E^T[m, n] (m on partitions), so
  exp(E^T) tiles feed the second matmul U[c, n] += vT.T @ P^T directly
  (PSUM-accumulated over m). The softmax denominator S[n] = sum_m P^T[m, n]
  is a ones-vector matmul accumulated in PSUM the same way.
- All matmul contractions are padded to K=128: half-array (K=64) matmuls
  keep the PE's HAM clock gate at 1.2 GHz; full-array streams run at 2.4.
- The E matmuls run 2 iterations ahead of the exp/U/S consumers (the PE is
  in-order; without the pipeline it stalls on ACT every iteration).
- bf16 operands throughout the attention math (fp32 PSUM accumulation,
  fp32 residual add); measured end-to-end rel err ~6e-3.
- No max-subtraction: logits are N(0, 64), |E| << 88 (fp32 exp overflow).
"""

import numpy as np

import concourse.bass as bass
import concourse.mybir as mybir
import concourse.tile as tile
from concourse import bacc
from concourse.bass_utils import run_bass_kernel_spmd

B = 4
CQK = 64
CV = 128
N = 4096
NQ = N // 2          # query columns per core
NG = 512             # n-group width (PSUM bank)
MT = 128             # m-tile height (PE contraction tile)
N_GROUPS = NQ // NG  # 4
N_MTILES = N // MT   # 32
VBLK = NG // MT      # vT-projection block = 4 m-tiles

F32 = mybir.dt.float32
F32R = mybir.dt.float32r
BF16 = mybir.dt.bfloat16


_last_results = None  # stashed BassKernelResults for test harnesses


def build_bass(gamma: float) -> bass.Bass:
    nc = bacc.Bacc()

    # xb rows CQK..127 are zero-padded on the host (full-K matmuls).
    xb = nc.declare_dram_parameter("xb", [MT, N], BF16, isOutput=False)
    xhb = nc.declare_dram_parameter("xhb", [CV, N], BF16, isOutput=False)
    xq = nc.declare_dram_parameter("xq", [CQK, NQ], BF16, isOutput=False)
    xh_res = nc.declare_dram_parameter("xh_res", [CV, NQ], F32, isOutput=False)
    aT = nc.declare_dram_parameter("aT", [CQK, CQK], BF16, isOutput=False)
    wvT = nc.declare_dram_parameter("wvT", [CV, CV], BF16, isOutput=False)
    o = nc.declare_dram_parameter("o", [CV, NQ], F32, isOutput=True)

    ts = bass.ts

    with tile.TileContext(nc) as tc:
        with (
            nc.allow_low_precision(reason="bf16 attention math, fp32 accum"),
            tc.tile_pool(name="const", bufs=1) as cpool,
            tc.tile_pool(name="pt", bufs=4) as ptpool,
            tc.tile_pool(name="ep", bufs=2, space="PSUM") as epool,
            tc.tile_pool(name="up", bufs=2, space="PSUM") as upool,
            tc.tile_pool(name="sp", bufs=1, space="PSUM") as spool,
            tc.tile_pool(name="mp", bufs=1, space="PSUM") as mpool,
            tc.tile_pool(name="out", bufs=3) as opool,
        ):
            # ---- persistent SBUF tensors ----
            xb_sb = cpool.tile([MT, N], BF16)
            xhb_sb = cpool.tile([CV, N], BF16)
            xq_sb = cpool.tile([CQK, NQ], BF16)
            xhres_sb = cpool.tile([CV, NQ], F32)
            aT_sb = cpool.tile([CQK, CQK], BF16)
            wvT_sb = cpool.tile([CV, CV], BF16)
            qk_sb = cpool.tile([MT, NQ], BF16)  # rows CQK..127 zero
            vT_sb = cpool.tile([CV, N], BF16)   # cols [mt*128,(mt+1)*128) = v[:, chunk].T
            ones_m = cpool.tile([MT, 1], BF16)  # S-matmul stationary
            ones_p = cpool.tile([1, CV], F32)   # gamma * ones: broadcast stationary
            zbias = cpool.tile([MT, 1], F32)

            # ---- loads, in consumer order ----
            nc.sync.dma_start(aT_sb[:], aT[:])
            nc.sync.dma_start(wvT_sb[:], wvT[:])
            for j in range(NQ // NG):
                nc.sync.dma_start(xq_sb[:, ts(j, NG)], xq[:, ts(j, NG)])
            for j in range(N // NG):
                nc.sync.dma_start(xhb_sb[:, ts(j, NG)], xhb[:, ts(j, NG)])
                nc.sync.dma_start(xb_sb[:, ts(j, NG)], xb[:, ts(j, NG)])
            for j in range(NQ // NG):
                nc.sync.dma_start(xhres_sb[:, ts(j, NG)], xh_res[:, ts(j, NG)])
            nc.gpsimd.memset(qk_sb[CQK:, :], 0.0)
            ones_stage = cpool.tile([MT, 1], F32)
            ones_stage2 = cpool.tile([1, CV], F32)
            nc.gpsimd.memset(ones_stage[:], 1.0)
            nc.gpsimd.memset(ones_stage2[:], float(gamma))
            nc.vector.tensor_copy(ones_m[:], ones_stage[:])
            nc.vector.tensor_copy(ones_p[:], ones_stage2[:])
            nc.gpsimd.memset(zbias[:], 0.0)

            # ---- qk projection: qk = (Wk^T Wq) @ xq ----
            for j in range(NQ // NG):
                qk_ps = epool.tile([CQK, NG], F32, tag="e", name=f"qkp_{j}")
                nc.tensor.matmul(qk_ps[:], aT_sb[:], xq_sb[:, ts(j, NG)],
                                 start=True, stop=True)
                nc.vector.tensor_copy(qk_sb[:CQK, ts(j, NG)], qk_ps[:])

            # ---- vT projection block j: vT[m, c] for m in [j*512,(j+1)*512) ----
            def emit_vblk(j):
                vt_ps = mpool.tile([CV, NG], F32, tag="mpsum", name=f"vtp_{j}")
                for u in range(VBLK):
                    mt = j * VBLK + u
                    nc.tensor.matmul(vt_ps[:, ts(u, MT)], xhb_sb[:, ts(mt, MT)],
                                     wvT_sb[:], start=True, stop=True)
                nc.vector.tensor_copy(vT_sb[:, ts(j, NG)], vt_ps[:])

            # ---- main flash loop (flat, software-pipelined, PAIRED) ----
            # E tiles come in [128, 1024] pairs (two m-tiles side by side, 2
            # PSUM banks) so one ACT instruction exps 1024 columns -- halves
            # the ACT instruction count, which is the binding engine.
            PIPE = 2          # pipeline depth in pairs
            NPAIRS_G = N_MTILES // 2
            NPT = N_GROUPS * NPAIRS_G

            def emit_Epair(g, pp):
                e2 = epool.tile([MT, 2 * NG], F32, tag="e", name=f"e_{g}_{pp}")
                nc.tensor.matmul(e2[:, :NG], xb_sb[:, ts(2 * pp, MT)],
                                 qk_sb[:, ts(g, NG)], start=True, stop=True)
                nc.tensor.matmul(e2[:, NG:], xb_sb[:, ts(2 * pp + 1, MT)],
                                 qk_sb[:, ts(g, NG)], start=True, stop=True)
                return e2

            def emit_epilogue(g, u_ps, s_ps):
                # out = gamma * U / S + x_h   (gamma baked into ones_p)
                r_sb = opool.tile([1, NG], F32, tag="r", name=f"r_{g}")
                nc.vector.reciprocal_approx_fast(out=r_sb[:], in_=s_ps[:1, :])
                rb_ps = mpool.tile([CV, NG], F32, tag="mpsum", name=f"rbp_{g}")
                nc.tensor.matmul(rb_ps[:], ones_p[:], r_sb[:],
                                 start=True, stop=True)
                rb_sb = opool.tile([CV, NG], F32, tag="rb", name=f"rb_{g}")
                nc.vector.tensor_copy(rb_sb[:], rb_ps[:])
                o_sb = opool.tile([CV, NG], F32, tag="o", name=f"o_{g}")
                nc.vector.tensor_mul(o_sb[:], u_ps[:], rb_sb[:])
                nc.vector.tensor_add(o_sb[:], o_sb[:], xhres_sb[:, ts(g, NG)])
                nc.sync.dma_start(o[:, ts(g, NG)], o_sb[:])

            def emit_S(s_ps, j, ptsum):
                nc.tensor.matmul(s_ps[:1, :], ones_m[:], ptsum[:],
                                 start=(j == 0), stop=(j == NPAIRS_G - 1))

            emit_vblk(0)
            emit_vblk(1)
            e_tiles = {p: emit_Epair(p // NPAIRS_G, p % NPAIRS_G)
                       for p in range(PIPE)}
            u_ps = s_ps = None
            pending = None
            pending_s = []
            for p in range(NPT):
                g, pp = divmod(p, NPAIRS_G)
                if pp == 0:
                    u_ps = upool.tile([CV, NG], F32, tag="u", name=f"u_{g}")
                    s_ps = spool.tile([1, NG], F32, tag="s", name=f"s_{g}")
                pt2 = ptpool.tile([MT, 2 * NG], BF16, tag="pt",
                                  name=f"pt_{g}_{pp}")
                nc.scalar.activation(pt2[:], e_tiles.pop(p)[:],
                                     mybir.ActivationFunctionType.Exp,
                                     bias=zbias[:])
                if p + PIPE < NPT:
                    gn, ppn = divmod(p + PIPE, NPAIRS_G)
                    e_tiles[p + PIPE] = emit_Epair(gn, ppn)
                if g == 0 and pp % 2 == 1 and pp // 2 + 2 < N // NG:
                    emit_vblk(pp // 2 + 2)
                lastp = pp == NPAIRS_G - 1
                # U[c, n] += vT_tile.T @ P^T  (both halves of the pair)
                nc.tensor.matmul(u_ps[:], vT_sb[:, ts(2 * pp, MT)],
                                 pt2[:, :NG], start=(pp == 0), stop=False)
                nc.tensor.matmul(u_ps[:], vT_sb[:, ts(2 * pp + 1, MT)],
                                 pt2[:, NG:], start=False, stop=lastp)
                if pending_s and pp >= 3:
                    for args in pending_s:
                        emit_S(s_ps, *args)
                    pending_s = []
                ptsum = ptpool.tile([MT, NG], BF16, tag="ptsum",
                                    name=f"ps_{g}_{pp}")
                nc.vector.tensor_add(ptsum[:], pt2[:, :NG], pt2[:, NG:])
                if lastp:
                    for args in pending_s:
                        emit_S(s_ps, *args)
                    pending_s = []
                    emit_S(s_ps, pp, ptsum)
                else:
                    pending_s.append((pp, ptsum))
                if pending is not None and (pp >= 1 or p == NPT - 1):
                    emit_epilogue(*pending)
                    pending = None
                if lastp:
                    pending = (g, u_ps, s_ps)
            emit_epilogue(*pending)

    nc.compile()
    return nc


def kernel(x, x_h, Wq, Wk, Wv, gamma):
    global _last_results
    import ml_dtypes
    bf16 = ml_dtypes.bfloat16

    x = np.ascontiguousarray(np.asarray(x, dtype=np.float32))
    x_h = np.ascontiguousarray(np.asarray(x_h, dtype=np.float32))
    Wq = np.asarray(Wq, dtype=np.float32)
    Wk = np.asarray(Wk, dtype=np.float32)
    Wv = np.asarray(Wv, dtype=np.float32)
    gval = float(np.asarray(gamma).reshape(-1)[0])

    nc = build_bass(gval)

    # qk = (Wk^T Wq) @ xq  ->  stationary operand is (Wk^T Wq)^T = Wq^T Wk
    aT = np.ascontiguousarray(Wq.T @ Wk).astype(bf16)
    wvT = np.ascontiguousarray(Wv.T).astype(bf16)
    x_bf = x.astype(bf16)
    xb_pad = np.zeros((B, MT, N), dtype=bf16)
    xb_pad[:, :CQK, :] = x_bf

    in_maps = []
    for core in range(8):
        b, h = core // 2, core % 2
        sl = slice(h * NQ, (h + 1) * NQ)
        in_maps.append({
            "xb": xb_pad[b],
            "xhb": x_h[b].astype(bf16),
            "xq": np.ascontiguousarray(x_bf[b][:, sl]),
            "xh_res": np.ascontiguousarray(x_h[b][:, sl]),
            "aT": aT,
            "wvT": wvT,
        })

    res = run_bass_kernel_spmd(nc, in_maps, list(range(8)))
    _last_results = res

    out = np.empty((B, CV, N), dtype=np.float32)
    for core in range(8):
        b, h = core // 2, core % 2
        out[b][:, h * NQ:(h + 1) * NQ] = res.results[core]["o"]
    return out



# revision 2
# speedup vs baseline: 1.0058x; 1.0058x over previous
"""Trainium2 Bass kernel for the CSA (channel-spatial attention) module.

Reference computation (per batch b):
    q = Wq @ x[b]            # [64, N]
    k = Wk @ x[b]            # [64, N]
    E[n, m] = sum_c q[c, n] * k[c, m]          # [N, N]
    A = softmax(E, axis=m)
    v = Wv @ x_h[b]          # [128, N]
    out[c, n] = sum_m v[c, m] * A[n, m]
    result = gamma * out + x_h[b]

Sharding: 8 cores = 4 batches x 2 query-halves. Each core holds full K/V for
its batch and a 2048-wide query chunk (flash-style: the [N, N] attention
matrix is never materialized in HBM).

Key transformations vs the naive mapping:
- Wk is folded into the query projection on the host:
  E^T[m, n] = sum_c' xb[c', m] * qk[c', n]  with  qk = (Wk^T Wq) @ x_chunk,
  so K needs no on-chip projection and the energy matmul consumes DMA'd
  x directly as its stationary operand.
- Energy is computed transposed, E^T[m, n] (m on partitions), so
  exp(E^T) tiles feed the second matmul U[c, n] += vT.T @ P^T directly
  (PSUM-accumulated over m).
- Softmax denominator S[n] = sum_m P^T[m, n] is accumulated OFF the PE:
  bf16 chained adds of the exp tiles on DVE (3 sub-chains per group) and
  GpSimd (1 sub-chain), partition-reduced by 2 ones-matmuls per group at
  the group end.  This removes 64 ptsum-adds' worth of S matmuls from the
  PE (the binding engine, ~15 us).
- All matmul contractions are padded to K=128: half-array (K=64) matmuls
  keep the PE's HAM clock gate at 1.2 GHz; full-array streams run at 2.4.
- The E matmuls run 2 iterations ahead of the exp/U consumers.
- bf16 operands throughout the attention math (fp32 PSUM accumulation,
  fp32 residual add).
- No max-subtraction: logits are N(0, 64), |E| << 88 (fp32 exp overflow).
"""

import numpy as np

import concourse.bass as bass
import concourse.mybir as mybir
import concourse.tile as tile
from concourse import bacc
from concourse.bass_utils import run_bass_kernel_spmd

B = 4
CQK = 64
CV = 128
N = 4096
NQ = N // 2          # query columns per core
NG = 512             # n-group width (PSUM bank)
MT = 128             # m-tile height (PE contraction tile)
N_GROUPS = NQ // NG  # 4
N_MTILES = N // MT   # 32
VBLK = NG // MT      # vT-projection block = 4 m-tiles

F32 = mybir.dt.float32
F32R = mybir.dt.float32r
BF16 = mybir.dt.bfloat16


_last_results = None  # stashed BassKernelResults for test harnesses


def build_bass(gamma: float) -> bass.Bass:
    nc = bacc.Bacc()

    # xb rows CQK..127 are zero-padded on the host (full-K matmuls).
    xb = nc.declare_dram_parameter("xb", [MT, N], BF16, isOutput=False)
    xhb = nc.declare_dram_parameter("xhb", [CV, N], BF16, isOutput=False)
    xq = nc.declare_dram_parameter("xq", [CQK, NQ], BF16, isOutput=False)
    xh_res = nc.declare_dram_parameter("xh_res", [CV, NQ], F32, isOutput=False)
    aT = nc.declare_dram_parameter("aT", [CQK, CQK], BF16, isOutput=False)
    wvT = nc.declare_dram_parameter("wvT", [CV, CV], BF16, isOutput=False)
    o = nc.declare_dram_parameter("o", [CV, NQ], F32, isOutput=True)

    ts = bass.ts

    with tile.TileContext(nc) as tc:
        with (
            nc.allow_low_precision(reason="bf16 attention math, fp32 accum"),
            tc.tile_pool(name="const", bufs=1) as cpool,
            tc.tile_pool(name="pt", bufs=4) as ptpool,
            tc.tile_pool(name="sacc", bufs=2) as sapool,
            tc.tile_pool(name="ep", bufs=2, space="PSUM") as epool,
            tc.tile_pool(name="up", bufs=2, space="PSUM") as upool,
            tc.tile_pool(name="sp", bufs=1, space="PSUM") as spool,
            tc.tile_pool(name="mp", bufs=1, space="PSUM") as mpool,
            tc.tile_pool(name="out", bufs=3) as opool,
        ):
            # ---- persistent SBUF tensors ----
            xb_sb = cpool.tile([MT, N], BF16)
            xhb_sb = cpool.tile([CV, N], BF16)
            xq_sb = cpool.tile([CQK, NQ], BF16)
            xhres_sb = cpool.tile([CV, NQ], F32)
            aT_sb = cpool.tile([CQK, CQK], BF16)
            wvT_sb = cpool.tile([CV, CV], BF16)
            qk_sb = cpool.tile([MT, NQ], BF16)  # rows CQK..127 zero
            vT_sb = cpool.tile([CV, N], BF16)   # cols [mt*128,(mt+1)*128) = v[:, chunk].T
            ones_m = cpool.tile([MT, 1], BF16)  # S-reduce matmul stationary
            ones_p = cpool.tile([1, CV], F32)   # gamma * ones: broadcast stationary
            zbias = cpool.tile([MT, 1], F32)

            # ---- loads: critical-path tensors first, bulk on gpsimd queue ----
            nc.sync.dma_start(aT_sb[:], aT[:])
            for j in range(NQ // NG):
                nc.sync.dma_start(xq_sb[:, ts(j, NG)], xq[:, ts(j, NG)])
            for j in range(N // NG):
                nc.sync.dma_start(xb_sb[:, ts(j, NG)], xb[:, ts(j, NG)])

            # gpsimd: small memsets first (zbias feeds the first ACT), then
            # issue the non-critical DMAs from its queue so the sync queue
            # stays short.
            nc.gpsimd.memset(zbias[:], 0.0)
            ones_stage = cpool.tile([MT, 1], F32)
            ones_stage2 = cpool.tile([1, CV], F32)
            nc.gpsimd.memset(ones_stage[:], 1.0)
            nc.gpsimd.memset(ones_stage2[:], float(gamma))
            nc.gpsimd.memset(qk_sb[CQK:, :], 0.0)
            nc.gpsimd.dma_start(wvT_sb[:], wvT[:])
            for j in range(N // NG):
                nc.gpsimd.dma_start(xhb_sb[:, ts(j, NG)], xhb[:, ts(j, NG)])
            for j in range(NQ // NG):
                nc.gpsimd.dma_start(xhres_sb[:, ts(j, NG)], xh_res[:, ts(j, NG)])
            nc.vector.tensor_copy(ones_m[:], ones_stage[:])
            nc.vector.tensor_copy(ones_p[:], ones_stage2[:])

            # ---- qk projection: qk = (Wk^T Wq) @ xq ----
            for j in range(NQ // NG):
                qk_ps = epool.tile([CQK, NG], F32, tag="e", name=f"qkp_{j}")
                nc.tensor.matmul(qk_ps[:], aT_sb[:], xq_sb[:, ts(j, NG)],
                                 start=True, stop=True)
                nc.vector.tensor_copy(qk_sb[:CQK, ts(j, NG)], qk_ps[:])

            # ---- vT projection block j: vT[m, c] for m in [j*512,(j+1)*512) ----
            def emit_vblk(j):
                vt_ps = mpool.tile([CV, NG], F32, tag="mpsum", name=f"vtp_{j}")
                for u in range(VBLK):
                    mt = j * VBLK + u
                    nc.tensor.matmul(vt_ps[:, ts(u, MT)], xhb_sb[:, ts(mt, MT)],
                                     wvT_sb[:], start=True, stop=True)
                nc.vector.tensor_copy(vT_sb[:, ts(j, NG)], vt_ps[:])

            # ---- main flash loop (flat, software-pipelined, PAIRED) ----
            # E tiles come in [128, 1024] pairs (two m-tiles side by side, 2
            # PSUM banks) so one ACT instruction exps 1024 columns.
            PIPE = 2          # pipeline depth in pairs
            NPAIRS_G = N_MTILES // 2
            NPT = N_GROUPS * NPAIRS_G
            HALF = NPAIRS_G // 2  # sub-chain length in pairs

            def emit_Epair(g, pp):
                e2 = epool.tile([MT, 2 * NG], F32, tag="e", name=f"e_{g}_{pp}")
                nc.tensor.matmul(e2[:, :NG], xb_sb[:, ts(2 * pp, MT)],
                                 qk_sb[:, ts(g, NG)], start=True, stop=True)
                nc.tensor.matmul(e2[:, NG:], xb_sb[:, ts(2 * pp + 1, MT)],
                                 qk_sb[:, ts(g, NG)], start=True, stop=True)
                return e2

            def emit_epilogue(g, u_ps, chains):
                # S[n] = sum_m P^T: combine bf16 sub-chains, partition-reduce
                # with two ones-matmuls into PSUM, then out = gamma*U/S + x_h.
                loA, loB, hiA, hiB = chains
                nc.vector.tensor_add(loA[:], loA[:], loB[:])
                nc.vector.tensor_add(hiA[:], hiA[:], hiB[:])
                s_ps = spool.tile([1, NG], F32, tag="s", name=f"s_{g}")
                nc.tensor.matmul(s_ps[:1, :], ones_m[:], loA[:],
                                 start=True, stop=False)
                nc.tensor.matmul(s_ps[:1, :], ones_m[:], hiA[:],
                                 start=False, stop=True)
                r_sb = opool.tile([1, NG], F32, tag="r", name=f"r_{g}")
                nc.vector.reciprocal_approx_fast(out=r_sb[:], in_=s_ps[:1, :])
                rb_ps = mpool.tile([CV, NG], F32, tag="mpsum", name=f"rbp_{g}")
                nc.tensor.matmul(rb_ps[:], ones_p[:], r_sb[:],
                                 start=True, stop=True)
                rb_sb = opool.tile([CV, NG], F32, tag="rb", name=f"rb_{g}")
                nc.vector.tensor_copy(rb_sb[:], rb_ps[:])
                o_sb = opool.tile([CV, NG], F32, tag="o", name=f"o_{g}")
                nc.vector.tensor_mul(o_sb[:], u_ps[:], rb_sb[:])
                nc.vector.tensor_add(o_sb[:], o_sb[:], xhres_sb[:, ts(g, NG)])
                nc.sync.dma_start(o[:, ts(g, NG)], o_sb[:])

            emit_vblk(0)
            emit_vblk(1)
            e_tiles = {p: emit_Epair(p // NPAIRS_G, p % NPAIRS_G)
                       for p in range(PIPE)}
            u_ps = None
            chains = None
            pending = None
            for p in range(NPT):
                g, pp = divmod(p, NPAIRS_G)
                if pp == 0:
                    u_ps = upool.tile([CV, NG], F32, tag="u", name=f"u_{g}")
                    # 4 bf16 S sub-chains: (lo/hi half) x (pairs 0-7 / 8-15)
                    chains = [sapool.tile([MT, NG], BF16, tag=f"sc{c}",
                                          name=f"sc{c}_{g}")
                              for c in range(4)]
                pt2 = ptpool.tile([MT, 2 * NG], BF16, tag="pt",
                                  name=f"pt_{g}_{pp}")
                nc.scalar.activation(pt2[:], e_tiles.pop(p)[:],
                                     mybir.ActivationFunctionType.Exp,
                                     bias=zbias[:])
                if p + PIPE < NPT:
                    gn, ppn = divmod(p + PIPE, NPAIRS_G)
                    e_tiles[p + PIPE] = emit_Epair(gn, ppn)
                if g == 0 and pp % 2 == 1 and pp // 2 + 2 < N // NG:
                    emit_vblk(pp // 2 + 2)
                lastp = pp == NPAIRS_G - 1
                # U[c, n] += vT_tile.T @ P^T  (both halves of the pair)
                nc.tensor.matmul(u_ps[:], vT_sb[:, ts(2 * pp, MT)],
                                 pt2[:, :NG], start=(pp == 0), stop=False)
                nc.tensor.matmul(u_ps[:], vT_sb[:, ts(2 * pp + 1, MT)],
                                 pt2[:, NG:], start=False, stop=lastp)
                # S sub-chain accumulation (off the PE): lo half on DVE,
                # hi half pairs 0-7 on GpSimd, pairs 8-15 on DVE.
                sub = pp // HALF           # 0 -> A-chain, 1 -> B-chain
                first = pp % HALF == 0
                lo, hi = chains[sub], chains[2 + sub]
                if first:
                    nc.vector.tensor_copy(lo[:], pt2[:, :NG])
                else:
                    nc.vector.tensor_add(lo[:], lo[:], pt2[:, :NG])
                hi_eng = nc.gpsimd if sub == 0 else nc.vector
                if first:
                    hi_eng.tensor_copy(hi[:], pt2[:, NG:])
                else:
                    hi_eng.tensor_add(hi[:], hi[:], pt2[:, NG:])
                if pending is not None and (pp >= 1 or p == NPT - 1):
                    emit_epilogue(*pending)
                    pending = None
                if lastp:
                    pending = (g, u_ps, chains)
            emit_epilogue(*pending)

    nc.compile()
    return nc


def kernel(x, x_h, Wq, Wk, Wv, gamma):
    global _last_results
    import ml_dtypes
    bf16 = ml_dtypes.bfloat16

    x = np.ascontiguousarray(np.asarray(x, dtype=np.float32))
    x_h = np.ascontiguousarray(np.asarray(x_h, dtype=np.float32))
    Wq = np.asarray(Wq, dtype=np.float32)
    Wk = np.asarray(Wk, dtype=np.float32)
    Wv = np.asarray(Wv, dtype=np.float32)
    gval = float(np.asarray(gamma).reshape(-1)[0])

    nc = build_bass(gval)

    # qk = (Wk^T Wq) @ xq  ->  stationary operand is (Wk^T Wq)^T = Wq^T Wk
    aT = np.ascontiguousarray(Wq.T @ Wk).astype(bf16)
    wvT = np.ascontiguousarray(Wv.T).astype(bf16)
    x_bf = x.astype(bf16)
    xb_pad = np.zeros((B, MT, N), dtype=bf16)
    xb_pad[:, :CQK, :] = x_bf

    in_maps = []
    for core in range(8):
        b, h = core // 2, core % 2
        sl = slice(h * NQ, (h + 1) * NQ)
        in_maps.append({
            "xb": xb_pad[b],
            "xhb": x_h[b].astype(bf16),
            "xq": np.ascontiguousarray(x_bf[b][:, sl]),
            "xh_res": np.ascontiguousarray(x_h[b][:, sl]),
            "aT": aT,
            "wvT": wvT,
        })

    res = run_bass_kernel_spmd(nc, in_maps, list(range(8)))
    _last_results = res

    out = np.empty((B, CV, N), dtype=np.float32)
    for core in range(8):
        b, h = core // 2, core % 2
        out[b][:, h * NQ:(h + 1) * NQ] = res.results[core]["o"]
    return out


# revision 3
# speedup vs baseline: 1.2193x; 1.2122x over previous
"""Trainium2 Bass kernel for the CSA (channel-spatial attention) module.

Reference computation (per batch b):
    q = Wq @ x[b]            # [64, N]
    k = Wk @ x[b]            # [64, N]
    E[n, m] = sum_c q[c, n] * k[c, m]          # [N, N]
    A = softmax(E, axis=m)
    v = Wv @ x_h[b]          # [128, N]
    out[c, n] = sum_m v[c, m] * A[n, m]
    result = gamma * out + x_h[b]

Sharding: 8 cores = 4 batches x 2 query-halves. Each core holds full K/V for
its batch and a 2048-wide query chunk (flash-style: the [N, N] attention
matrix is never materialized in HBM).

Key transformations vs the naive mapping:
- Wk is folded into the query projection on the host:
  E^T[m, n] = sum_c' xb[c', m] * qk[c', n]  with  qk = (Wk^T Wq) @ x_chunk.
- Energy is computed transposed, E^T[m, n] (m on partitions), so
  exp(E^T) tiles feed the second matmul U[c, n] += vT.T @ P^T directly
  (PSUM-accumulated over m).
- Softmax denominator S[n]: exp tiles are accumulated in bf16 on the DVE
  (two 8-pair chains per group, full 1024-wide adds), then partition-
  reduced by matmuls against an ALL-ONES [128,128] stationary, which
  lands S replicated across all 128 PSUM partitions.  The reciprocal and
  the U*(1/S) multiply then run full-width on DVE with no broadcast
  matmul (the K=1 broadcast matmul also triggered HAM half-array
  throttle bursts).  gamma is folded into the ones stationary.
- GpSimd does only memsets + bulk DMA issue: its tensor ops contend with
  DVE for SBUF ports and slow both engines down.
- All matmul contractions are K=128 full-array.
- The E matmuls run 2 iterations ahead of the exp/U consumers; each
  group's epilogue PE work is deferred into the next group so the
  in-order PE queue never waits on DVE.
- bf16 operands throughout the attention math (fp32 PSUM accumulation,
  fp32 residual add).
- No max-subtraction: logits are N(0, 64), |E| << 88 (fp32 exp overflow).
"""

import numpy as np

import concourse.bass as bass
import concourse.mybir as mybir
import concourse.tile as tile
from concourse import bacc
from concourse.bass_utils import run_bass_kernel_spmd

B = 4
CQK = 64
CV = 128
N = 4096
NQ = N // 2          # query columns per core
NG = 512             # n-group width (PSUM bank)
MT = 128             # m-tile height (PE contraction tile)
N_GROUPS = NQ // NG  # 4
N_MTILES = N // MT   # 32
VBLK = NG // MT      # vT-projection block = 4 m-tiles

F32 = mybir.dt.float32
BF16 = mybir.dt.bfloat16


_last_results = None  # stashed BassKernelResults for test harnesses


def build_bass(gamma: float) -> bass.Bass:
    nc = bacc.Bacc()

    # xb rows CQK..127 are zero-padded on the host (full-K matmuls).
    xb = nc.declare_dram_parameter("xb", [MT, N], BF16, isOutput=False)
    xhb = nc.declare_dram_parameter("xhb", [CV, N], BF16, isOutput=False)
    xq = nc.declare_dram_parameter("xq", [CQK, NQ], BF16, isOutput=False)
    xh_res = nc.declare_dram_parameter("xh_res", [CV, NQ], F32, isOutput=False)
    aT = nc.declare_dram_parameter("aT", [CQK, CQK], BF16, isOutput=False)
    wvT = nc.declare_dram_parameter("wvT", [CV, CV], BF16, isOutput=False)
    o = nc.declare_dram_parameter("o", [CV, NQ], F32, isOutput=True)

    ts = bass.ts

    with tile.TileContext(nc) as tc:
        with (
            nc.allow_low_precision(reason="bf16 attention math, fp32 accum"),
            tc.tile_pool(name="const", bufs=1) as cpool,
            tc.tile_pool(name="pt", bufs=4) as ptpool,
            tc.tile_pool(name="sacc", bufs=2) as sapool,
            tc.tile_pool(name="ep", bufs=2, space="PSUM") as epool,
            tc.tile_pool(name="up", bufs=2, space="PSUM") as upool,
            tc.tile_pool(name="sp", bufs=1, space="PSUM") as spool,
            tc.tile_pool(name="mp", bufs=1, space="PSUM") as mpool,
            tc.tile_pool(name="out", bufs=3) as opool,
        ):
            # ---- persistent SBUF tensors ----
            xb_sb = cpool.tile([MT, N], BF16)
            xhb_sb = cpool.tile([CV, N], BF16)
            xq_sb = cpool.tile([CQK, NQ], BF16)
            xhres_sb = cpool.tile([CV, NQ], F32)
            aT_sb = cpool.tile([CQK, CQK], BF16)
            wvT_sb = cpool.tile([CV, CV], BF16)
            qk_sb = cpool.tile([MT, NQ], BF16)  # rows CQK..127 zero
            vT_sb = cpool.tile([CV, N], BF16)   # cols [mt*128,(mt+1)*128) = v[:, chunk].T
            ones_g = cpool.tile([MT, MT], BF16)  # gamma * all-ones (S-reduce)
            zbias = cpool.tile([MT, 1], F32)

            # ---- loads: critical-path tensors first, bulk on gpsimd queue ----
            nc.sync.dma_start(aT_sb[:], aT[:])
            nc.sync.dma_start(xq_sb[:], xq[:])
            for j in range(N // NG):
                nc.sync.dma_start(xb_sb[:, ts(j, NG)], xb[:, ts(j, NG)])

            # gpsimd: memsets (qk pad feeds the first E matmul), then issue
            # the non-critical DMAs from its queue so the sync queue stays
            # short.  No gpsimd tensor work: it contends with DVE SBUF ports.
            nc.gpsimd.memset(qk_sb[CQK:, :], 0.0)
            nc.gpsimd.memset(zbias[:], 0.0)
            ones_stage = cpool.tile([MT, MT], F32)
            nc.gpsimd.memset(ones_stage[:], 1.0)
            nc.gpsimd.dma_start(wvT_sb[:], wvT[:])
            for j in range(N // NG):
                nc.gpsimd.dma_start(xhb_sb[:, ts(j, NG)], xhb[:, ts(j, NG)])
            for j in range(NQ // NG):
                nc.gpsimd.dma_start(xhres_sb[:, ts(j, NG)], xh_res[:, ts(j, NG)])
            # gamma folded into the S-reduce stationary:
            # s_ps = sum_m gamma*P  =>  1/s... NO: s must be exact sum.
            # gamma goes into the reciprocal instead: r = gamma / S, done by
            # scaling ones by 1/1 and folding gamma into recip via multiply.
            nc.vector.tensor_copy(ones_g[:], ones_stage[:])

            # ---- qk projection: qk = (Wk^T Wq) @ xq ----
            # PSUM->SBUF bf16 copies on the scalar engine (idle at startup).
            for j in range(NQ // NG):
                qk_ps = epool.tile([CQK, NG], F32, tag="e", name=f"qkp_{j}")
                nc.tensor.matmul(qk_ps[:], aT_sb[:], xq_sb[:, ts(j, NG)],
                                 start=True, stop=True)
                nc.scalar.copy(qk_sb[:CQK, ts(j, NG)], qk_ps[:])

            # ---- vT projection block j: vT[m, c] for m in [j*512,(j+1)*512) ----
            def emit_vblk(j):
                vt_ps = mpool.tile([CV, NG], F32, tag="mpsum", name=f"vtp_{j}")
                for u in range(VBLK):
                    mt = j * VBLK + u
                    nc.tensor.matmul(vt_ps[:, ts(u, MT)], xhb_sb[:, ts(mt, MT)],
                                     wvT_sb[:], start=True, stop=True)
                nc.vector.tensor_copy(vT_sb[:, ts(j, NG)], vt_ps[:])

            # ---- main flash loop (flat, software-pipelined, PAIRED) ----
            PIPE = 2          # pipeline depth in pairs
            NPAIRS_G = N_MTILES // 2
            NPT = N_GROUPS * NPAIRS_G
            HALF = NPAIRS_G // 2  # sub-chain length in pairs

            def emit_Epair(g, pp):
                e2 = epool.tile([MT, 2 * NG], F32, tag="e", name=f"e_{g}_{pp}")
                nc.tensor.matmul(e2[:, :NG], xb_sb[:, ts(2 * pp, MT)],
                                 qk_sb[:, ts(g, NG)], start=True, stop=True)
                nc.tensor.matmul(e2[:, NG:], xb_sb[:, ts(2 * pp + 1, MT)],
                                 qk_sb[:, ts(g, NG)], start=True, stop=True)
                return e2

            def emit_sreduce(s_ps, chain, first, last):
                # s_ps[i, n] = sum over chains/halves of P^T -- replicated on
                # all 128 partitions via the all-ones stationary.
                nc.tensor.matmul(s_ps[:], ones_g[:], chain[:, :NG],
                                 start=first, stop=False)
                nc.tensor.matmul(s_ps[:], ones_g[:], chain[:, NG:],
                                 start=False, stop=last)

            def emit_epilogue(g, u_ps, s_ps, chainB, gamma):
                # finish S with the B-chain, then out = gamma * U / S + x_h
                emit_sreduce(s_ps, chainB, first=False, last=True)
                r_sb = opool.tile([CV, NG], F32, tag="r", name=f"r_{g}")
                nc.vector.reciprocal_approx_fast(out=r_sb[:], in_=s_ps[:])
                o_sb = opool.tile([CV, NG], F32, tag="o", name=f"o_{g}")
                nc.vector.tensor_mul(o_sb[:], u_ps[:], r_sb[:])
                nc.vector.scalar_tensor_tensor(
                    out=o_sb[:], in0=o_sb[:], scalar=gamma,
                    in1=xhres_sb[:, ts(g, NG)],
                    op0=mybir.AluOpType.mult, op1=mybir.AluOpType.add)
                nc.sync.dma_start(o[:, ts(g, NG)], o_sb[:])

            emit_vblk(0)
            emit_vblk(1)
            e_tiles = {p: emit_Epair(p // NPAIRS_G, p % NPAIRS_G)
                       for p in range(PIPE)}
            u_ps = None
            s_ps = None
            chains = None
            pending = None
            for p in range(NPT):
                g, pp = divmod(p, NPAIRS_G)
                if pp == 0:
                    u_ps = upool.tile([CV, NG], F32, tag="u", name=f"u_{g}")
                    s_ps = spool.tile([CV, NG], F32, tag="s", name=f"s_{g}")
                    # 2 full-width bf16 S chains: pairs 0-7 and 8-15
                    chains = [sapool.tile([MT, 2 * NG], BF16, tag=f"sc{c}",
                                          name=f"sc{c}_{g}")
                              for c in range(2)]
                pt2 = ptpool.tile([MT, 2 * NG], BF16, tag="pt",
                                  name=f"pt_{g}_{pp}")
                nc.scalar.activation(pt2[:], e_tiles.pop(p)[:],
                                     mybir.ActivationFunctionType.Exp,
                                     bias=zbias[:])
                if p + PIPE < NPT:
                    gn, ppn = divmod(p + PIPE, NPAIRS_G)
                    e_tiles[p + PIPE] = emit_Epair(gn, ppn)
                if g == 0 and pp % 2 == 1 and pp // 2 + 2 < N // NG:
                    emit_vblk(pp // 2 + 2)
                lastp = pp == NPAIRS_G - 1
                # U[c, n] += vT_tile.T @ P^T  (both halves of the pair)
                nc.tensor.matmul(u_ps[:], vT_sb[:, ts(2 * pp, MT)],
                                 pt2[:, :NG], start=(pp == 0), stop=False)
                nc.tensor.matmul(u_ps[:], vT_sb[:, ts(2 * pp + 1, MT)],
                                 pt2[:, NG:], start=False, stop=lastp)
                # S chain accumulation on DVE (bf16, full 1024-wide)
                sub = pp // HALF
                chain = chains[sub]
                if pp % HALF == 0:
                    nc.vector.tensor_copy(chain[:], pt2[:])
                else:
                    nc.vector.tensor_add(chain[:], chain[:], pt2[:])
                # A-chain finished at pp==7: fold it into s_ps mid-group.
                if pp == HALF + 2:
                    emit_sreduce(s_ps, chains[0], first=True, last=False)
                # previous group's epilogue, deferred to pp>=5 so the PE
                # queue never blocks on the DVE chain tail.
                if pending is not None and (pp >= 5 or p == NPT - 1):
                    emit_epilogue(*pending)
                    pending = None
                if lastp:
                    pending = (g, u_ps, s_ps, chains[1], gamma)
            emit_epilogue(*pending)

    nc.compile()
    return nc


def kernel(x, x_h, Wq, Wk, Wv, gamma):
    global _last_results
    import ml_dtypes
    bf16 = ml_dtypes.bfloat16

    x = np.ascontiguousarray(np.asarray(x, dtype=np.float32))
    x_h = np.ascontiguousarray(np.asarray(x_h, dtype=np.float32))
    Wq = np.asarray(Wq, dtype=np.float32)
    Wk = np.asarray(Wk, dtype=np.float32)
    Wv = np.asarray(Wv, dtype=np.float32)
    gval = float(np.asarray(gamma).reshape(-1)[0])

    nc = build_bass(gval)

    # qk = (Wk^T Wq) @ xq  ->  stationary operand is (Wk^T Wq)^T = Wq^T Wk
    aT = np.ascontiguousarray(Wq.T @ Wk).astype(bf16)
    wvT = np.ascontiguousarray(Wv.T).astype(bf16)
    x_bf = x.astype(bf16)
    xb_pad = np.zeros((B, MT, N), dtype=bf16)
    xb_pad[:, :CQK, :] = x_bf

    in_maps = []
    for core in range(8):
        b, h = core // 2, core % 2
        sl = slice(h * NQ, (h + 1) * NQ)
        in_maps.append({
            "xb": xb_pad[b],
            "xhb": x_h[b].astype(bf16),
            "xq": np.ascontiguousarray(x_bf[b][:, sl]),
            "xh_res": np.ascontiguousarray(x_h[b][:, sl]),
            "aT": aT,
            "wvT": wvT,
        })

    res = run_bass_kernel_spmd(nc, in_maps, list(range(8)))
    _last_results = res

    out = np.empty((B, CV, N), dtype=np.float32)
    for core in range(8):
        b, h = core // 2, core % 2
        out[b][:, h * NQ:(h + 1) * NQ] = res.results[core]["o"]
    return out
